# revision 7
# baseline (speedup 1.0000x reference)
"""Trainium2 Bass kernel for nn_DuelingDQN (2-layer LSTM + dueling-advantage MLP).

Strategy
--------
Data-parallel over batch: B=4096 is split as 512 per NeuronCore across 8 cores;
weights are replicated. On each core everything is kept in a transposed layout
(features on the SBUF partition dim, batch on the free dim), so the sequential
T=100 recurrence runs as a chain of bf16 matmuls (fp32 PSUM accumulation):

    gates.T (1024 x 512) = W.T-slices.T @ [h.T ...]

256-row feature tensors (h, c, per-gate activations) are stored "folded" as
(128, 2, 512) SBUF tiles — the middle dim indexes feature rows [0:128) /
[128:256) — which halves the elementwise-op count.

The layer-0 input projection W_ih0 @ x_t + b0 has no recurrence dependency, so
it is computed on the host (fp32, more accurate than a device bf16 matmul) and
streamed in as per-step (128, 8, 512) bf16 tiles; on device a single DVE add
folds each gate's x-contribution into the hh0 PSUM accumulation. This removes
~10 matmul instructions per step from the PE, which is the bottleneck engine
(sustained N=512 matmul costs ~283 ns on this stack vs the 213 ns streaming
floor, so instruction count is what matters).

Per step: PE does 54 matmuls; gate sigmoid/tanh on ACT straight out of PSUM;
cell updates on DVE (c stays fp32, h is written as bf16 for the next matmul).
Layer-1 emits its recurrent matmuls for a gate pair before the input matmuls,
and the MLP head for step t is emitted in the middle of step t+1, so PE keeps
streaming across the h0/h1 ACT+DVE tails of the recurrence.

BIR post-processing (engine streams are in-order, so both rewrites preserve
semantics):
  - `_strip_redundant_sync` drops same-engine semaphore waits and dead
    updates (the Tile framework emits a sem-inc per matmul; only the ones a
    cross-engine consumer actually waits on are kept, ~15 ns/matmul saved).
  - `_split_multiwaits` hoists extra waits onto injected same-engine
    EventSemaphore instructions (this walrus build encodes at most ONE
    sync-wait per instruction).
"""

import json
import sys
import types
from contextlib import ExitStack

import numpy as np

sys.path.insert(0, "/opt/trn_rl_repo")

import ml_dtypes  # noqa: E402

N_CORES = 8
B, T, IN, H = 4096, 100, 140, 256
BC = B // N_CORES  # 512 batch per core
G = 4 * H  # 1024 gate rows
BF16 = ml_dtypes.bfloat16
F16 = np.float16


# --------------------------------------------------------------------------
# BIR post-processing (see module docstring)
# --------------------------------------------------------------------------
def _block_chain_is_linear(fn) -> bool:
    blocks = fn["blocks"]
    names = [b.get("name") for b in blocks]
    for i, b in enumerate(blocks[:-1]):
        insts = b["instructions"]
        term = insts[-1] if insts else None
        if term is None or term.get("opcode") != "UnconditionalBranch":
            if term is not None and term.get("opcode") in ("ConditionalBranch",):
                return False
            continue
        tgt = term.get("target") or term.get("dest") or term.get("target_block")
        if tgt is not None and names[i + 1] is not None and tgt != names[i + 1]:
            return False
    return True


def _strip_redundant_sync(bir: dict) -> dict:
    stats = {"waits_dropped": 0, "updates_dropped": 0, "sems": 0}
    for fn in bir["functions"]:
        if not _block_chain_is_linear(fn):
            continue
        all_insts = []
        for blk in fn["blocks"]:
            all_insts.extend(blk["instructions"])
        upd = {}
        waits = {}
        for ins in all_insts:
            si = ins.get("sync_info")
            if not si:
                continue
            for u in si.get("on_update") or []:
                if u.get("sync_type") == "semaphore":
                    upd.setdefault(u["id"], []).append((ins, ins.get("engine"), u))
            for w in si.get("on_wait") or []:
                if w.get("sync_type") == "semaphore":
                    waits.setdefault(w["id"], []).append((ins, ins.get("engine"), w))
        for sem, ups in upd.items():
            engines = {e for _, e, _ in ups}
            if len(engines) != 1:
                continue
            (eng,) = engines
            if eng in (None, "", "SP"):
                continue
            if not all(u.get("update_mode") == "sem-inc" and
                       u.get("update_value") == 1 for _, _, u in ups):
                continue
            wl = waits.get(sem, [])
            if not all(w.get("wait_mode") == "sem-ge-imm" for _, _, w in wl):
                continue
            stats["sems"] += 1
            cross = []
            for ins, weng, w in wl:
                if weng == eng:
                    si = ins["sync_info"]
                    si["on_wait"] = [x for x in si["on_wait"] if x is not w]
                    stats["waits_dropped"] += 1
                else:
                    cross.append(w)
            thresholds = sorted({w["wait_value"] for w in cross})
            if thresholds and thresholds[-1] > len(ups):
                continue
            rank = {t: k + 1 for k, t in enumerate(thresholds)}
            keep = set(thresholds)
            for pos, (ins, _, u) in enumerate(ups, start=1):
                if pos not in keep:
                    si = ins["sync_info"]
                    si["on_update"] = [x for x in si["on_update"] if x is not u]
                    stats["updates_dropped"] += 1
            for w in cross:
                w["wait_value"] = rank[w["wait_value"]]
    return stats


def _split_multiwaits(bir: dict) -> int:
    ctr = 0
    for f in bir["functions"]:
        for blk in f["blocks"]:
            new_insts = []
            for ins in blk["instructions"]:
                si = ins.get("sync_info")
                waits = (si or {}).get("on_wait") or []
                if len(waits) > 1:
                    for w in waits[:-1]:
                        ctr += 1
                        new_insts.append(
                            {
                                "debug": ins.get("debug", 0),
                                "engine": ins["engine"],
                                "ins": [],
                                "outs": [],
                                "name": f"antsplitw-{ctr}",
                                "opcode": "EventSemaphore",
                                "sync_info": {"on_update": [], "on_wait": [w]},
                            }
                        )
                    si["on_wait"] = [waits[-1]]
                new_insts.append(ins)
            blk["instructions"] = new_insts
    return ctr


def _patch_bass(nc, strip=True):
    import concourse.mybir as mybir

    def to_json_bytes(self):
        j = json.loads(mybir.module_to_json_bytes(self.m))
        if strip:
            _strip_redundant_sync(j)
        _split_multiwaits(j)
        return json.dumps(j).encode()

    nc.to_json_bytes = types.MethodType(to_json_bytes, nc)
    return nc


# --------------------------------------------------------------------------
# Module build
# --------------------------------------------------------------------------
def build_module(b_a2_val: float, T_steps: int = T, opts: dict | None = None):
    opts = opts or {}
    import concourse.bass as bass
    import concourse.tile as tile
    from concourse import mybir

    f32 = mybir.dt.float32
    f16 = mybir.dt.float16
    AF = mybir.ActivationFunctionType

    nc = bass.Bass("TRN2", target_bir_lowering=False, debug=False)

    # host-precomputed layer-0 input projection, per step: (128, 8, 512)
    # where [p, m, b] = (W_ih0 @ x_t + b0)[128*m + p, batch b]
    # t_data < T_steps wraps xg reads (timing-only configs)
    t_data = opts.get("t_data", T_steps)
    xg_d = nc.dram_tensor("xg", (t_data, 128, 8, BC), f16,
                          kind="ExternalInput").ap()
    whh0_d = nc.dram_tensor("whh0", (128, 2, G), f16, kind="ExternalInput").ap()
    wih1_d = nc.dram_tensor("wih1", (128, 2, G), f16, kind="ExternalInput").ap()
    whh1_d = nc.dram_tensor("whh1", (128, 2, G), f16, kind="ExternalInput").ap()
    wa1_d = nc.dram_tensor("wa1", (128, 2, H), f16, kind="ExternalInput").ap()
    wa2_d = nc.dram_tensor("wa2", (128, 2), f16, kind="ExternalInput").ap()
    bias1_d = nc.dram_tensor("bias1", (128, 8), f32, kind="ExternalInput").ap()
    ba1_d = nc.dram_tensor("ba1", (128, 2), f32, kind="ExternalInput").ap()
    o_d = nc.dram_tensor("o", (T_steps, BC), f32, kind="ExternalOutput").ap()

    GATE_FUNCS = [AF.Sigmoid, AF.Sigmoid, AF.Tanh, AF.Sigmoid]  # i, f, g, o

    with tile.TileContext(nc) as tc, ExitStack() as ctx:
        persist = ctx.enter_context(tc.tile_pool(name="persist", bufs=1))
        xpool = ctx.enter_context(tc.tile_pool(name="xpool", bufs=opts.get("xbufs", 3)))
        gpool = ctx.enter_context(tc.tile_pool(name="gates_sb", bufs=opts.get("gbufs", 3)))
        tpool = ctx.enter_context(tc.tile_pool(name="tmp_sb", bufs=opts.get("tbufs", 3)))
        psg = ctx.enter_context(
            tc.tile_pool(name="ps_gates", bufs=opts.get("psbufs", 3), space="PSUM"))
        if opts.get("psbufs", 3) == 3:
            pso = ctx.enter_context(tc.tile_pool(name="ps_out", bufs=2, space="PSUM"))
        else:
            pso = psg  # mlp2 output shares the gates slots (frees 2 banks)

        def load(name, dram_ap, shape, dt):
            t = persist.tile(shape, dt, tag=name, name=name)
            nc.sync.dma_start(t[:], dram_ap)
            return t

        whh0 = load("whh0", whh0_d[:], [128, 2, G], f16)
        wih1 = load("wih1", wih1_d[:], [128, 2, G], f16)
        whh1 = load("whh1", whh1_d[:], [128, 2, G], f16)
        wa1 = load("wa1", wa1_d[:], [128, 2, H], f16)
        wa2 = load("wa2", wa2_d[:], [128, 2], f16)
        bias1 = load("bias1", bias1_d[:], [128, 8], f32)
        ba1 = load("ba1", ba1_d[:], [128, 2], f32)

        h0 = persist.tile([128, 2, BC], f16, tag="h0", name="h0")
        h1 = persist.tile([128, 2, BC], f16, tag="h1", name="h1")
        c0 = persist.tile([128, 2, BC], f16, tag="c0", name="c0")
        c1 = persist.tile([128, 2, BC], f16, tag="c1", name="c1")

        def make_tiles(t, lname, g):
            ps = psg.tile([128, 2, BC], f32, tag="gates", name=f"ps_{lname}{g}_{t}")
            sb = gpool.tile([128, 2, BC], f16, tag=f"g{g}",
                            name=f"sb_{lname}{g}_{t}")
            return ps, sb

        def cell(t, lname, gates, h, c):
            gi, gf, gg, go = gates
            if t > 0:
                # c*f first: it only waits on the f-gate ACT (ready mid-layer)
                t1 = tpool.tile([128, 2, BC], f16, tag="t1", name=f"t1_{lname}_{t}")
                for j in range(2):
                    nc.vector.tensor_mul(c[:, j, :], c[:, j, :], gf[:, j, :])
                    nc.vector.tensor_mul(t1[:, j, :], gi[:, j, :], gg[:, j, :])
                    nc.vector.tensor_add(c[:, j, :], c[:, j, :], t1[:, j, :])
            else:
                nc.vector.tensor_mul(c[:], gi[:], gg[:])
            # halves: the consumer's first matmul only needs one h half early
            tc_t = tpool.tile([128, 2, BC], f16, tag="tanhc", name=f"tc_{lname}_{t}")
            for j in range(2):
                nc.scalar.activation(tc_t[:, j, :], c[:, j, :], AF.Tanh)
                nc.vector.tensor_mul(h[:, j, :], go[:, j, :], tc_t[:, j, :])

        def l0_layer(t, xg, mid_hook=None):
            """Layer 0 for step t: hh0 matmuls accumulate in PSUM, then one
            DVE add folds in the host-computed x-projection (incl. b0), then
            ACT applies the gate nonlinearity."""
            gates = [None] * 4
            for q in (0, 1):
                pair = (2 * q, 2 * q + 1)
                tiles = {}
                for g in pair:
                    tiles[g] = make_tiles(t, "l0", g)
                    gates[g] = tiles[g][1]
                for g in pair:
                    ps = tiles[g][0]
                    for j in range(2):
                        col = 128 * (2 * g + j)
                        if t > 0:
                            for k in range(2):
                                nc.tensor.matmul(
                                    ps[:, j, :],
                                    whh0[:, k, col : col + 128],
                                    h0[:, k, :],
                                    start=(k == 0), stop=(k == 1),
                                )
                for g in pair:
                    ps, sb = tiles[g]
                    if t > 0:
                        nc.vector.tensor_add(ps[:], ps[:], xg[:, 2 * g : 2 * g + 2, :])
                        nc.scalar.activation(sb[:], ps[:], GATE_FUNCS[g])
                    else:
                        nc.scalar.activation(sb[:], xg[:, 2 * g : 2 * g + 2, :],
                                             GATE_FUNCS[g])
            if mid_hook is not None:
                # MLP[t-1] ACT/PE work lands between the gate ACTs and the
                # cell's tanh, so ACT's in-order queue isn't head-of-line
                # blocked waiting for the DVE c-chain
                mid_hook()
            cell(t, "l0", gates, h0, c0)

        def l1_layer(t):
            """Layer 1 with gate-2's hh matmuls hoisted between pair-0's hh
            and ih blocks: extra PE cover before the h0[t]-dependent ih
            matmuls (max 3 live PSUM tensors throughout)."""
            tiles = {}
            gates = [None] * 4

            def alloc(g):
                tiles[g] = make_tiles(t, "l1", g)
                gates[g] = tiles[g][1]

            def hh(g):
                for j in range(2):
                    col = 128 * (2 * g + j)
                    out = tiles[g][0][:, j, :]
                    for k in range(2):
                        nc.tensor.matmul(
                            out, whh1[:, k, col : col + 128], h1[:, k, :],
                            start=(k == 0), stop=False,
                        )

            def ih_act(g):
                ps, sb = tiles[g]
                for j in range(2):
                    m = 2 * g + j
                    col = 128 * m
                    out = ps[:, j, :]
                    for k in range(2):
                        nc.tensor.matmul(
                            out, wih1[:, k, col : col + 128], h0[:, k, :],
                            start=(t == 0 and k == 0), stop=(k == 1),
                        )
                    nc.scalar.activation(sb[:, j, :], out, GATE_FUNCS[g],
                                         bias=bias1[:, m : m + 1])

            if t > 0:
                alloc(0); alloc(1)
                hh(0); hh(1)
                alloc(2); hh(2)
                ih_act(0); ih_act(1)
                alloc(3); hh(3)
                ih_act(2); ih_act(3)
            else:
                for g in range(4):
                    alloc(g)
                for g in range(4):
                    ih_act(g)
            cell(t, "l1", gates, h1, c1)

        def mlp_head(t):
            """Advantage head for step t; reads current h1 contents."""
            ps_a = psg.tile([128, 2, BC], f32, tag="gates", name=f"ps_a1_{t}")
            relu = tpool.tile([128, 2, BC], f16, tag="relu", name=f"relu_{t}")
            for j in range(2):
                out = ps_a[:, j, :]
                for k in range(2):
                    nc.tensor.matmul(
                        out, wa1[:, k, 128 * j : 128 * j + 128], h1[:, k, :],
                        start=(k == 0), stop=(k == 1),
                    )
                nc.scalar.activation(relu[:, j, :], out, AF.Relu,
                                     bias=ba1[:, j : j + 1])
            po_tag = "gates" if pso is psg else "po"
            ps_o = pso.tile([1, BC], f32, tag=po_tag, name=f"ps_o_{t}")
            for k in range(2):
                nc.tensor.matmul(ps_o[:], wa2[:, k : k + 1], relu[:, k, :],
                                 start=(k == 0), stop=(k == 1))
            # b_a2 is added on the host
            osb = tpool.tile([1, BC], f32, tag="osb", name=f"osb_{t}")
            nc.vector.tensor_copy(osb[:], ps_o[:])
            nc.sync.dma_start(o_d[t : t + 1, :], osb[:])

        xgs = {}
        for t in range(T_steps):
            xg = xpool.tile([128, 8, BC], f16, tag="xg", name=f"xg_{t}")
            nc.sync.dma_start(xg[:], xg_d[t % t_data])
            xgs[t] = xg

            l0_layer(t, xg,
                     mid_hook=(lambda tt=t: mlp_head(tt - 1)) if t > 0 else None)
            l1_layer(t)
            xgs.pop(t - 2, None)
        mlp_head(T_steps - 1)

    return _patch_bass(nc, strip=not opts.get("nostrip"))


# --------------------------------------------------------------------------
# Host-side input prep / output assembly
# --------------------------------------------------------------------------
def _fold3(wT: np.ndarray) -> np.ndarray:
    """(2K, M) -> (128, 2, M): middle dim indexes K-rows [0:128) / [128:256)."""
    k2, m = wT.shape
    assert k2 == 256
    return np.ascontiguousarray(wT.reshape(2, 128, m).transpose(1, 0, 2))


def prepare_in_maps(inputs: dict) -> list[dict]:
    f32 = np.float32
    W_ih0 = np.asarray(inputs["W_ih0"], f32)
    W_hh0 = np.asarray(inputs["W_hh0"], f32)
    W_ih1 = np.asarray(inputs["W_ih1"], f32)
    W_hh1 = np.asarray(inputs["W_hh1"], f32)
    W_a1 = np.asarray(inputs["W_a1"], f32)
    W_a2 = np.asarray(inputs["W_a2"], f32)

    b0 = np.asarray(inputs["b_ih0"], f32) + np.asarray(inputs["b_hh0"], f32)
    b1 = np.asarray(inputs["b_ih1"], f32) + np.asarray(inputs["b_hh1"], f32)

    shared = {
        "whh0": _fold3(W_hh0.T).astype(F16),
        "wih1": _fold3(W_ih1.T).astype(F16),
        "whh1": _fold3(W_hh1.T).astype(F16),
        "wa1": _fold3(W_a1.T).astype(F16),
        "wa2": np.ascontiguousarray(
            W_a2.reshape(2, 128).T.reshape(128, 2)).astype(F16),
        "bias1": np.ascontiguousarray(b1.reshape(8, 128).T),
        "ba1": np.ascontiguousarray(np.asarray(inputs["b_a1"], f32)
                                    .reshape(2, 128).T),
    }

    # Layer-0 input projection on the host (fp32 — beats a device bf16
    # matmul on accuracy): xg[t, p, m, b] = (W_ih0 @ x[b,t] + b0)[128m + p]
    x = np.asarray(inputs["x"], f32)  # (B, T, IN)
    t_steps = x.shape[1]
    xf = np.ascontiguousarray(x.reshape(B * t_steps, IN))  # (B*T, IN)
    g = xf @ W_ih0.T.astype(f32)  # (B*T, G)
    g += b0
    # (B, T, G) -> (T, G, B) -> (T, 8, 128, B) -> (T, 128, 8, B)
    g = g.reshape(B, t_steps, G).transpose(1, 2, 0)
    g = np.ascontiguousarray(
        g.reshape(t_steps, 8, 128, B).transpose(0, 2, 1, 3)).astype(F16)

    in_maps = []
    for c in range(N_CORES):
        in_maps.append(
            {"xg": np.ascontiguousarray(g[:, :, :, c * BC : (c + 1) * BC]),
             **shared})
    return in_maps


def assemble_output(results: list[dict], b_a2_val: float) -> np.ndarray:
    out_tb = np.concatenate([r["o"] for r in results], axis=1)  # (T, B)
    out_tb = out_tb + np.float32(b_a2_val)
    t_steps = out_tb.shape[0]
    return np.ascontiguousarray(out_tb.reshape(B, t_steps))


_module_cache: dict = {}


def get_module(b_a2_val: float):
    key = round(float(b_a2_val), 12)
    if key not in _module_cache:
        _module_cache[key] = build_module(float(b_a2_val))
    return _module_cache[key]


def kernel(**inputs) -> np.ndarray:
    from concourse import bass_utils

    b_a2_val = float(np.asarray(inputs["b_a2"], np.float32).reshape(-1)[0])
    nc = get_module(b_a2_val)
    in_maps = prepare_in_maps(inputs)
    res = bass_utils.run_bass_kernel_spmd(nc, in_maps, core_ids=list(range(N_CORES)))
    return assemble_output(res.results, b_a2_val)
